# revision 17
# baseline (speedup 1.0000x reference)
"""MoE top-2 routing kernel for 8 TRN2 NeuronCores.

Strategy (expert-parallel, per the sharding hint):
  - Host computes the tiny gate (T x E logits, top-2 + softmax) and the
    aux load-balancing loss, and builds per-expert token lists.
  - Token dispatch ("all-to-all") happens during input sharding: core e
    receives the tokens routed to expert e (gathered, transposed, padded
    to a uniform capacity C) plus expert e's weights, pre-tiled into
    [128,128] matmul blocks.
  - Each core runs a dense 2-layer GELU FFN over its C token slots:
        h = gelu(x @ w1[e] + b1[e]);  y = h @ w2[e]
    with f32r matmuls (full-rate fp32 mode on the PE array).
  - Unshard = weighted scatter-add on host: out[t] += g[t,e] * (y + b2[e]).

The device program is identical on all 8 cores (SPMD); all per-core
variation lives in the input data. Capacity C is derived from the actual
routing at call time; compiled programs are cached per C.
"""

import sys

sys.path.insert(0, "/opt/trn_rl_repo")

import numpy as np

D_MODEL = 1024
D_FF = 4096
NUM_EXPERTS = 8
TOP_K = 2
N_CORES = 8
KD = D_MODEL // 128   # 8  k-chunks of the d_model contraction
MF = D_FF // 128      # 32 m-chunks of the d_ff dimension
DD = D_MODEL // 128   # 8  d-chunks of the output dim (as 2 halves of 4)

_CACHE = {}


def _token_tiles(C):
    """Split capacity C into moving-dim tiles (512s, then one 256)."""
    tiles = [512] * (C // 512)
    if C % 512:
        assert C % 512 == 256
        tiles.append(256)
    return tiles


def _build(C):
    import concourse.bass as bass  # noqa: F401
    import concourse.mybir as mybir
    import concourse.tile as tile
    from concourse import bacc

    F32 = mybir.dt.float32
    F32R = mybir.dt.float32r
    AF = mybir.ActivationFunctionType

    nc = bacc.Bacc(None, target_bir_lowering=False)

    xgT = nc.dram_tensor("xgT", [KD, 128, C], F32R, kind="ExternalInput")
    w1tt = nc.dram_tensor("w1tt", [MF, 128, KD, 128], F32R, kind="ExternalInput")
    w2tt = nc.dram_tensor("w2tt", [DD, 128, MF, 128], F32R, kind="ExternalInput")
    b1t = nc.dram_tensor("b1t", [128, MF], F32, kind="ExternalInput")
    yT = nc.dram_tensor("yT", [DD, 128, C], F32, kind="ExternalOutput")

    tiles = _token_tiles(C)

    with tile.TileContext(nc) as tc:
        with (
            tc.tile_pool(name="consts", bufs=1) as consts,
            tc.tile_pool(name="xpool", bufs=2) as xpool,
            tc.tile_pool(name="wpool", bufs=8) as wpool,
            tc.tile_pool(name="w2pool", bufs=2) as w2pool,
            tc.tile_pool(name="hpool", bufs=1) as hpool,
            tc.tile_pool(name="ypool", bufs=3) as ypool,
            tc.tile_pool(name="ps1", bufs=3, space="PSUM") as ps1,
            tc.tile_pool(name="ps2", bufs=2, space="PSUM") as ps2,
        ):
            b1sb = consts.tile([128, MF], F32)
            nc.sync.dma_start(out=b1sb, in_=b1t[:, :])

            t0 = 0
            for nt in tiles:
                # ---- load this tile's activations: [128, KD, nt] ----
                xts = xpool.tile([128, KD, 512], F32R, tag="xts")
                nc.sync.dma_start(
                    out=xts[:, :, :nt],
                    in_=xgT[:, :, t0 : t0 + nt].rearrange("k p n -> p k n"),
                )

                # ---- phase A: h[m] = gelu(sum_k w1[k,m]^T x[k] + b1[m]) ----
                h = hpool.tile([128, MF, 512], F32R, tag="h")
                for m in range(MF):
                    w1sb = wpool.tile([128, KD, 128], F32R, tag="w1sb")
                    nc.sync.dma_start(out=w1sb, in_=w1tt[m])
                    psum1 = ps1.tile([128, 512], F32, tag="psum1")
                    for k in range(KD):
                        nc.tensor.matmul(
                            psum1[:, :nt],
                            lhsT=w1sb[:, k, :],
                            rhs=xts[:, k, :nt],
                            start=(k == 0),
                            stop=(k == KD - 1),
                        )
                    nc.scalar.activation(
                        h[:, m, :nt],
                        psum1[:, :nt],
                        AF.Gelu,
                        bias=b1sb[:, m : m + 1],
                    )

                # ---- phase B: y[dq] = sum_m w2[m,dq]^T h[m], one dq sweep ----
                for dq in range(DD):
                    w2sb = w2pool.tile([128, MF, 128], F32R, tag="w2sb")
                    nc.sync.dma_start(out=w2sb, in_=w2tt[dq])
                    psum2 = ps2.tile([128, 512], F32, tag="psum2")
                    for m in range(MF):
                        nc.tensor.matmul(
                            psum2[:, :nt],
                            lhsT=w2sb[:, m, :],
                            rhs=h[:, m, :nt],
                            start=(m == 0),
                            stop=(m == MF - 1),
                        )
                    y_sb = ypool.tile([128, 512], F32, tag="y_sb")
                    nc.vector.tensor_copy(y_sb[:, :nt], psum2[:, :nt])
                    nc.sync.dma_start(
                        out=yT[dq, :, t0 : t0 + nt],
                        in_=y_sb[:, :nt],
                    )
                t0 += nt

    nc.compile()
    return nc


def _get_program(C):
    if C not in _CACHE:
        _CACHE[C] = _build(C)
    return _CACHE[C]


def kernel(x, gate_w, gate_b, w1, b1, w2, b2):
    from concourse.bass_utils import run_bass_kernel_spmd

    x = np.asarray(x, dtype=np.float32)
    gate_w = np.asarray(gate_w, dtype=np.float32)
    gate_b = np.asarray(gate_b, dtype=np.float32)
    w1 = np.asarray(w1, dtype=np.float32)
    b1 = np.asarray(b1, dtype=np.float32)
    w2 = np.asarray(w2, dtype=np.float32)
    b2 = np.asarray(b2, dtype=np.float32)

    B, S, D = x.shape
    T = B * S
    xf = np.ascontiguousarray(x.reshape(T, D))

    # ---- gate (host): logits, top-2, softmax over the 2, aux loss ----
    logits = xf @ gate_w + gate_b                       # [T, E] f32
    sel = np.argsort(-logits, axis=-1, kind="stable")[:, :TOP_K]
    tl = np.take_along_axis(logits, sel, axis=1).astype(np.float64)
    ex = np.exp(tl - tl.max(axis=1, keepdims=True))
    topw = (ex / ex.sum(axis=1, keepdims=True)).astype(np.float32)

    counts = np.bincount(sel.ravel(), minlength=NUM_EXPERTS)
    fraction = counts.astype(np.float64) / (T * TOP_K)
    l64 = logits.astype(np.float64)
    sm = np.exp(l64 - l64.max(axis=1, keepdims=True))
    sm /= sm.sum(axis=1, keepdims=True)
    mean_prob = sm.mean(axis=0)
    aux_loss = np.float32(NUM_EXPERTS * np.sum(fraction * mean_prob))

    # ---- dispatch: gather each expert's tokens, pad to capacity C ----
    C = max(256, int(-(-counts.max() // 256)) * 256)
    nc = _get_program(C)

    sel0 = sel[:, 0]
    routing = []
    in_maps = []
    for e in range(NUM_EXPERTS):
        m0 = sel0 == e
        m_any = m0 | (sel[:, 1] == e)
        idx = np.nonzero(m_any)[0]
        g = np.where(m0[idx], topw[idx, 0], topw[idx, 1]).astype(np.float32)
        routing.append((idx, g))

        xg = np.zeros((C, D_MODEL), dtype=np.float32)
        xg[: len(idx)] = xf[idx]
        xgT = np.ascontiguousarray(xg.T).reshape(KD, 128, C)

        w1tt = np.ascontiguousarray(
            w1[e].reshape(KD, 128, MF, 128).transpose(2, 1, 0, 3)
        )
        w2tt = np.ascontiguousarray(
            w2[e].reshape(MF, 128, DD, 128).transpose(2, 1, 0, 3)
        )
        b1t = np.ascontiguousarray(b1[e].reshape(MF, 128).T)
        in_maps.append({"xgT": xgT, "w1tt": w1tt, "w2tt": w2tt, "b1t": b1t})

    res = run_bass_kernel_spmd(nc, in_maps, list(range(N_CORES)))

    # ---- unshard: weighted scatter-add (+ b2), experts in ascending order ----
    out = np.zeros((T, D_MODEL), dtype=np.float32)
    for e in range(NUM_EXPERTS):
        idx, g = routing[e]
        y = res.results[e]["yT"].reshape(D_MODEL, C)[:, : len(idx)].T
        out[idx] += g[:, None] * (y + b2[e])

    return out.reshape(B, S, D_MODEL), aux_loss


# revision 18
# speedup vs baseline: 1.0211x; 1.0211x over previous
"""MoE top-2 routing kernel for 8 TRN2 NeuronCores.

Strategy (expert-parallel, per the sharding hint):
  - Host computes the tiny gate (T x E logits, top-2 + softmax) and the
    aux load-balancing loss, and builds per-expert token lists.
  - Token dispatch ("all-to-all") happens during input sharding: core e
    receives the tokens routed to expert e (gathered, transposed, padded
    to a uniform capacity C) plus expert e's weights, pre-tiled into
    [128,128] matmul blocks.
  - Each core runs a dense 2-layer GELU FFN over its C token slots:
        h = gelu(x @ w1[e] + b1[e]);  y = h @ w2[e]
    with f32r matmuls (full-rate fp32 mode on the PE array).
  - Unshard = weighted scatter-add on host: out[t] += g[t,e] * (y + b2[e]).

The device program is identical on all 8 cores (SPMD); all per-core
variation lives in the input data. Capacity C is derived from the actual
routing at call time; compiled programs are cached per C.
"""

import sys

sys.path.insert(0, "/opt/trn_rl_repo")

import numpy as np

D_MODEL = 1024
D_FF = 4096
NUM_EXPERTS = 8
TOP_K = 2
N_CORES = 8
KD = D_MODEL // 128   # 8  k-chunks of the d_model contraction
MF = D_FF // 128      # 32 m-chunks of the d_ff dimension
DD = D_MODEL // 128   # 8  d-chunks of the output dim (as 2 halves of 4)

_CACHE = {}


def _token_tiles(C):
    """Split capacity C into moving-dim tiles (512s, then one 256)."""
    tiles = [512] * (C // 512)
    if C % 512:
        assert C % 512 == 256
        tiles.append(256)
    return tiles


def _build(C):
    import concourse.bass as bass  # noqa: F401
    import concourse.mybir as mybir
    import concourse.tile as tile
    from concourse import bacc

    F32 = mybir.dt.float32
    F32R = mybir.dt.float32r
    AF = mybir.ActivationFunctionType

    nc = bacc.Bacc(None, target_bir_lowering=False)

    xgT = nc.dram_tensor("xgT", [KD, 128, C], F32R, kind="ExternalInput")
    w1tt = nc.dram_tensor("w1tt", [MF, 128, KD, 128], F32R, kind="ExternalInput")
    w2tt = nc.dram_tensor("w2tt", [DD, 128, MF, 128], F32R, kind="ExternalInput")
    b1t = nc.dram_tensor("b1t", [128, MF], F32, kind="ExternalInput")
    yT = nc.dram_tensor("yT", [DD, 128, C], F32, kind="ExternalOutput")

    tiles = _token_tiles(C)

    with tile.TileContext(nc) as tc:
        with (
            tc.tile_pool(name="consts", bufs=1) as consts,
            tc.tile_pool(name="xpool", bufs=2) as xpool,
            tc.tile_pool(name="wpool", bufs=8) as wpool,
            tc.tile_pool(name="w2pool", bufs=2) as w2pool,
            tc.tile_pool(name="hpool", bufs=1) as hpool,
            tc.tile_pool(name="ypool", bufs=3) as ypool,
            tc.tile_pool(name="ps1", bufs=3, space="PSUM") as ps1,
            tc.tile_pool(name="ps2", bufs=2, space="PSUM") as ps2,
        ):
            b1sb = consts.tile([128, MF], F32)
            nc.sync.dma_start(out=b1sb, in_=b1t[:, :])

            t0 = 0
            for nt in tiles:
                # ---- load this tile's activations: [128, KD, nt] ----
                xts = xpool.tile([128, KD, 512], F32R, tag="xts")
                nc.sync.dma_start(
                    out=xts[:, :, :nt],
                    in_=xgT[:, :, t0 : t0 + nt].rearrange("k p n -> p k n"),
                )

                # ---- phase A: h[m] = gelu(sum_k w1[k,m]^T x[k] + b1[m]) ----
                h = hpool.tile([128, MF, 512], F32R, tag="h")
                for m in range(MF):
                    w1sb = wpool.tile([128, KD, 128], F32R, tag="w1sb")
                    nc.sync.dma_start(out=w1sb, in_=w1tt[m])
                    psum1 = ps1.tile([128, 512], F32, tag="psum1")
                    for k in range(KD):
                        nc.tensor.matmul(
                            psum1[:, :nt],
                            lhsT=w1sb[:, k, :],
                            rhs=xts[:, k, :nt],
                            start=(k == 0),
                            stop=(k == KD - 1),
                        )
                    nc.scalar.activation(
                        h[:, m, :nt],
                        psum1[:, :nt],
                        AF.Gelu,
                        bias=b1sb[:, m : m + 1],
                    )

                # ---- phase B: y[dq] = sum_m w2[m,dq]^T h[m], one dq sweep ----
                for dq in range(DD):
                    w2sb = w2pool.tile([128, MF, 128], F32R, tag="w2sb")
                    nc.sync.dma_start(
                        out=w2sb[:, : MF // 2, :], in_=w2tt[dq, :, : MF // 2, :]
                    )
                    nc.sync.dma_start(
                        out=w2sb[:, MF // 2 :, :], in_=w2tt[dq, :, MF // 2 :, :]
                    )
                    psum2 = ps2.tile([128, 512], F32, tag="psum2")
                    for m in range(MF):
                        nc.tensor.matmul(
                            psum2[:, :nt],
                            lhsT=w2sb[:, m, :],
                            rhs=h[:, m, :nt],
                            start=(m == 0),
                            stop=(m == MF - 1),
                        )
                    y_sb = ypool.tile([128, 512], F32, tag="y_sb")
                    nc.vector.tensor_copy(y_sb[:, :nt], psum2[:, :nt])
                    nc.sync.dma_start(
                        out=yT[dq, :, t0 : t0 + nt],
                        in_=y_sb[:, :nt],
                    )
                t0 += nt

    nc.compile()
    return nc


def _get_program(C):
    if C not in _CACHE:
        _CACHE[C] = _build(C)
    return _CACHE[C]


def kernel(x, gate_w, gate_b, w1, b1, w2, b2):
    from concourse.bass_utils import run_bass_kernel_spmd

    x = np.asarray(x, dtype=np.float32)
    gate_w = np.asarray(gate_w, dtype=np.float32)
    gate_b = np.asarray(gate_b, dtype=np.float32)
    w1 = np.asarray(w1, dtype=np.float32)
    b1 = np.asarray(b1, dtype=np.float32)
    w2 = np.asarray(w2, dtype=np.float32)
    b2 = np.asarray(b2, dtype=np.float32)

    B, S, D = x.shape
    T = B * S
    xf = np.ascontiguousarray(x.reshape(T, D))

    # ---- gate (host): logits, top-2, softmax over the 2, aux loss ----
    logits = xf @ gate_w + gate_b                       # [T, E] f32
    sel = np.argsort(-logits, axis=-1, kind="stable")[:, :TOP_K]
    tl = np.take_along_axis(logits, sel, axis=1).astype(np.float64)
    ex = np.exp(tl - tl.max(axis=1, keepdims=True))
    topw = (ex / ex.sum(axis=1, keepdims=True)).astype(np.float32)

    counts = np.bincount(sel.ravel(), minlength=NUM_EXPERTS)
    fraction = counts.astype(np.float64) / (T * TOP_K)
    l64 = logits.astype(np.float64)
    sm = np.exp(l64 - l64.max(axis=1, keepdims=True))
    sm /= sm.sum(axis=1, keepdims=True)
    mean_prob = sm.mean(axis=0)
    aux_loss = np.float32(NUM_EXPERTS * np.sum(fraction * mean_prob))

    # ---- dispatch: gather each expert's tokens, pad to capacity C ----
    C = max(256, int(-(-counts.max() // 256)) * 256)
    nc = _get_program(C)

    sel0 = sel[:, 0]
    routing = []
    in_maps = []
    for e in range(NUM_EXPERTS):
        m0 = sel0 == e
        m_any = m0 | (sel[:, 1] == e)
        idx = np.nonzero(m_any)[0]
        g = np.where(m0[idx], topw[idx, 0], topw[idx, 1]).astype(np.float32)
        routing.append((idx, g))

        xg = np.zeros((C, D_MODEL), dtype=np.float32)
        xg[: len(idx)] = xf[idx]
        xgT = np.ascontiguousarray(xg.T).reshape(KD, 128, C)

        w1tt = np.ascontiguousarray(
            w1[e].reshape(KD, 128, MF, 128).transpose(2, 1, 0, 3)
        )
        w2tt = np.ascontiguousarray(
            w2[e].reshape(MF, 128, DD, 128).transpose(2, 1, 0, 3)
        )
        b1t = np.ascontiguousarray(b1[e].reshape(MF, 128).T)
        in_maps.append({"xgT": xgT, "w1tt": w1tt, "w2tt": w2tt, "b1t": b1t})

    res = run_bass_kernel_spmd(nc, in_maps, list(range(N_CORES)))

    # ---- unshard: weighted scatter-add (+ b2), experts in ascending order ----
    out = np.zeros((T, D_MODEL), dtype=np.float32)
    for e in range(NUM_EXPERTS):
        idx, g = routing[e]
        y = res.results[e]["yT"].reshape(D_MODEL, C)[:, : len(idx)].T
        out[idx] += g[:, None] * (y + b2[e])

    return out.reshape(B, S, D_MODEL), aux_loss


# revision 19
# speedup vs baseline: 1.0437x; 1.0222x over previous
"""MoE top-2 routing kernel for 8 TRN2 NeuronCores.

Strategy (expert-parallel, per the sharding hint):
  - Host computes the tiny gate (T x E logits, top-2 + softmax) and the
    aux load-balancing loss, and builds per-expert token lists.
  - Token dispatch ("all-to-all") happens during input sharding: core e
    receives the tokens routed to expert e (gathered, transposed, padded
    to a uniform capacity C) plus expert e's weights, pre-tiled into
    [128,128] matmul blocks.
  - Each core runs a dense 2-layer GELU FFN over its C token slots:
        h = gelu(x @ w1[e] + b1[e]);  y = h @ w2[e]
    with f32r matmuls (full-rate fp32 mode on the PE array).
  - Unshard = weighted scatter-add on host: out[t] += g[t,e] * (y + b2[e]).

The device program is identical on all 8 cores (SPMD); all per-core
variation lives in the input data. Capacity C is derived from the actual
routing at call time; compiled programs are cached per C.
"""

import sys

sys.path.insert(0, "/opt/trn_rl_repo")

import numpy as np

D_MODEL = 1024
D_FF = 4096
NUM_EXPERTS = 8
TOP_K = 2
N_CORES = 8
KD = D_MODEL // 128   # 8  k-chunks of the d_model contraction
MF = D_FF // 128      # 32 m-chunks of the d_ff dimension
DD = D_MODEL // 128   # 8  d-chunks of the output dim (as 2 halves of 4)

_CACHE = {}


def _token_tiles(C):
    """Split capacity C into moving-dim tiles (512s, then one 256)."""
    tiles = [512] * (C // 512)
    if C % 512:
        assert C % 512 == 256
        tiles.append(256)
    return tiles


def _build(C):
    import concourse.bass as bass  # noqa: F401
    import concourse.mybir as mybir
    import concourse.tile as tile
    from concourse import bacc

    F32 = mybir.dt.float32
    F32R = mybir.dt.float32r
    AF = mybir.ActivationFunctionType

    nc = bacc.Bacc(None, target_bir_lowering=False)

    xgT = nc.dram_tensor("xgT", [KD, 128, C], F32R, kind="ExternalInput")
    w1tt = nc.dram_tensor("w1tt", [MF, 128, KD, 128], F32R, kind="ExternalInput")
    w2tt = nc.dram_tensor("w2tt", [DD, 128, MF, 128], F32R, kind="ExternalInput")
    b1t = nc.dram_tensor("b1t", [128, MF], F32, kind="ExternalInput")
    yT = nc.dram_tensor("yT", [DD, 128, C], F32, kind="ExternalOutput")

    tiles = _token_tiles(C)

    with tile.TileContext(nc) as tc:
        with (
            tc.tile_pool(name="consts", bufs=1) as consts,
            tc.tile_pool(name="xpool", bufs=2) as xpool,
            tc.tile_pool(name="wpool", bufs=8) as wpool,
            tc.tile_pool(name="w2pool", bufs=3) as w2pool,
            tc.tile_pool(name="hpool", bufs=1) as hpool,
            tc.tile_pool(name="ypool", bufs=3) as ypool,
            tc.tile_pool(name="ps1", bufs=3, space="PSUM") as ps1,
            tc.tile_pool(name="ps2", bufs=2, space="PSUM") as ps2,
        ):
            b1sb = consts.tile([128, MF], F32)
            nc.sync.dma_start(out=b1sb, in_=b1t[:, :])

            t0 = 0
            for nt in tiles:
                # ---- load this tile's activations: [128, KD, nt] ----
                xts = xpool.tile([128, KD, 512], F32R, tag="xts")
                nc.sync.dma_start(
                    out=xts[:, :, :nt],
                    in_=xgT[:, :, t0 : t0 + nt].rearrange("k p n -> p k n"),
                )

                # ---- phase A: h[m] = gelu(sum_k w1[k,m]^T x[k] + b1[m]) ----
                h = hpool.tile([128, MF, 512], F32R, tag="h")
                for m in range(MF):
                    w1sb = wpool.tile([128, KD, 128], F32R, tag="w1sb")
                    nc.sync.dma_start(out=w1sb, in_=w1tt[m])
                    psum1 = ps1.tile([128, 512], F32, tag="psum1")
                    for k in range(KD):
                        nc.tensor.matmul(
                            psum1[:, :nt],
                            lhsT=w1sb[:, k, :],
                            rhs=xts[:, k, :nt],
                            start=(k == 0),
                            stop=(k == KD - 1),
                        )
                    nc.scalar.activation(
                        h[:, m, :nt],
                        psum1[:, :nt],
                        AF.Gelu,
                        bias=b1sb[:, m : m + 1],
                    )

                # ---- phase B: y[dq] = sum_m w2[m,dq]^T h[m], one dq sweep ----
                for dq in range(DD):
                    w2sb = w2pool.tile([128, MF, 128], F32R, tag="w2sb")
                    nc.sync.dma_start(
                        out=w2sb[:, : MF // 2, :], in_=w2tt[dq, :, : MF // 2, :]
                    )
                    nc.sync.dma_start(
                        out=w2sb[:, MF // 2 :, :], in_=w2tt[dq, :, MF // 2 :, :]
                    )
                    psum2 = ps2.tile([128, 512], F32, tag="psum2")
                    for m in range(MF):
                        nc.tensor.matmul(
                            psum2[:, :nt],
                            lhsT=w2sb[:, m, :],
                            rhs=h[:, m, :nt],
                            start=(m == 0),
                            stop=(m == MF - 1),
                        )
                    y_sb = ypool.tile([128, 512], F32, tag="y_sb")
                    nc.vector.tensor_copy(y_sb[:, :nt], psum2[:, :nt])
                    nc.sync.dma_start(
                        out=yT[dq, :, t0 : t0 + nt],
                        in_=y_sb[:, :nt],
                    )
                t0 += nt

    nc.compile()
    return nc


def _get_program(C):
    if C not in _CACHE:
        _CACHE[C] = _build(C)
    return _CACHE[C]


def kernel(x, gate_w, gate_b, w1, b1, w2, b2):
    from concourse.bass_utils import run_bass_kernel_spmd

    x = np.asarray(x, dtype=np.float32)
    gate_w = np.asarray(gate_w, dtype=np.float32)
    gate_b = np.asarray(gate_b, dtype=np.float32)
    w1 = np.asarray(w1, dtype=np.float32)
    b1 = np.asarray(b1, dtype=np.float32)
    w2 = np.asarray(w2, dtype=np.float32)
    b2 = np.asarray(b2, dtype=np.float32)

    B, S, D = x.shape
    T = B * S
    xf = np.ascontiguousarray(x.reshape(T, D))

    # ---- gate (host): logits, top-2, softmax over the 2, aux loss ----
    logits = xf @ gate_w + gate_b                       # [T, E] f32
    sel = np.argsort(-logits, axis=-1, kind="stable")[:, :TOP_K]
    tl = np.take_along_axis(logits, sel, axis=1).astype(np.float64)
    ex = np.exp(tl - tl.max(axis=1, keepdims=True))
    topw = (ex / ex.sum(axis=1, keepdims=True)).astype(np.float32)

    counts = np.bincount(sel.ravel(), minlength=NUM_EXPERTS)
    fraction = counts.astype(np.float64) / (T * TOP_K)
    l64 = logits.astype(np.float64)
    sm = np.exp(l64 - l64.max(axis=1, keepdims=True))
    sm /= sm.sum(axis=1, keepdims=True)
    mean_prob = sm.mean(axis=0)
    aux_loss = np.float32(NUM_EXPERTS * np.sum(fraction * mean_prob))

    # ---- dispatch: gather each expert's tokens, pad to capacity C ----
    C = max(256, int(-(-counts.max() // 256)) * 256)
    nc = _get_program(C)

    sel0 = sel[:, 0]
    routing = []
    in_maps = []
    for e in range(NUM_EXPERTS):
        m0 = sel0 == e
        m_any = m0 | (sel[:, 1] == e)
        idx = np.nonzero(m_any)[0]
        g = np.where(m0[idx], topw[idx, 0], topw[idx, 1]).astype(np.float32)
        routing.append((idx, g))

        xg = np.zeros((C, D_MODEL), dtype=np.float32)
        xg[: len(idx)] = xf[idx]
        xgT = np.ascontiguousarray(xg.T).reshape(KD, 128, C)

        w1tt = np.ascontiguousarray(
            w1[e].reshape(KD, 128, MF, 128).transpose(2, 1, 0, 3)
        )
        w2tt = np.ascontiguousarray(
            w2[e].reshape(MF, 128, DD, 128).transpose(2, 1, 0, 3)
        )
        b1t = np.ascontiguousarray(b1[e].reshape(MF, 128).T)
        in_maps.append({"xgT": xgT, "w1tt": w1tt, "w2tt": w2tt, "b1t": b1t})

    res = run_bass_kernel_spmd(nc, in_maps, list(range(N_CORES)))

    # ---- unshard: weighted scatter-add (+ b2), experts in ascending order ----
    out = np.zeros((T, D_MODEL), dtype=np.float32)
    for e in range(NUM_EXPERTS):
        idx, g = routing[e]
        y = res.results[e]["yT"].reshape(D_MODEL, C)[:, : len(idx)].T
        out[idx] += g[:, None] * (y + b2[e])

    return out.reshape(B, S, D_MODEL), aux_loss


# revision 21
# speedup vs baseline: 1.0631x; 1.0186x over previous
"""MoE top-2 routing kernel for 8 TRN2 NeuronCores.

Strategy (expert-parallel, per the sharding hint):
  - Host computes the tiny gate (T x E logits, top-2 + softmax) and the
    aux load-balancing loss, and builds per-expert token lists.
  - Token dispatch ("all-to-all") happens during input sharding: core e
    receives the tokens routed to expert e (gathered, transposed, padded
    to a uniform capacity C) plus expert e's weights, pre-tiled into
    [128,128] matmul blocks.
  - Each core runs a dense 2-layer GELU FFN over its C token slots:
        h = gelu(x @ w1[e] + b1[e]);  y = h @ w2[e]
    with f32r matmuls (full-rate fp32 mode on the PE array).
  - Unshard = weighted scatter-add on host: out[t] += g[t,e] * (y + b2[e]).

The device program is identical on all 8 cores (SPMD); all per-core
variation lives in the input data. Capacity C is derived from the actual
routing at call time; compiled programs are cached per C.
"""

import sys

sys.path.insert(0, "/opt/trn_rl_repo")

import numpy as np

D_MODEL = 1024
D_FF = 4096
NUM_EXPERTS = 8
TOP_K = 2
N_CORES = 8
KD = D_MODEL // 128   # 8  k-chunks of the d_model contraction
MF = D_FF // 128      # 32 m-chunks of the d_ff dimension
DD = D_MODEL // 128   # 8  d-chunks of the output dim (as 2 halves of 4)

_CACHE = {}


def _token_tiles(C):
    """Split capacity C into moving-dim tiles (512s, then one 256)."""
    tiles = [512] * (C // 512)
    if C % 512:
        assert C % 512 == 256
        tiles.append(256)
    return tiles


def _build(C):
    import concourse.bass as bass  # noqa: F401
    import concourse.mybir as mybir
    import concourse.tile as tile
    from concourse import bacc

    F32 = mybir.dt.float32
    F32R = mybir.dt.float32r
    AF = mybir.ActivationFunctionType

    nc = bacc.Bacc(None, target_bir_lowering=False)

    xgT = nc.dram_tensor("xgT", [KD, 128, C], F32R, kind="ExternalInput")
    w1tt = nc.dram_tensor("w1tt", [MF, 128, KD, 128], F32R, kind="ExternalInput")
    w2tt = nc.dram_tensor("w2tt", [DD, 128, MF, 128], F32R, kind="ExternalInput")
    b1t = nc.dram_tensor("b1t", [128, MF], F32, kind="ExternalInput")
    yT = nc.dram_tensor("yT", [DD, 128, C], F32, kind="ExternalOutput")

    tiles = _token_tiles(C)

    with tile.TileContext(nc) as tc:
        with (
            tc.tile_pool(name="consts", bufs=1) as consts,
            tc.tile_pool(name="xpool", bufs=2) as xpool,
            tc.tile_pool(name="wpool", bufs=8) as wpool,
            tc.tile_pool(name="w2pool", bufs=3) as w2pool,
            tc.tile_pool(name="hpool", bufs=1) as hpool,
            tc.tile_pool(name="ypool", bufs=3) as ypool,
            tc.tile_pool(name="ps1", bufs=4, space="PSUM") as ps1,
            tc.tile_pool(name="ps2", bufs=3, space="PSUM") as ps2,
        ):
            b1sb = consts.tile([128, MF], F32)
            nc.sync.dma_start(out=b1sb, in_=b1t[:, :])

            t0 = 0
            for nt in tiles:
                # ---- load this tile's activations: [128, KD, nt] ----
                xts = xpool.tile([128, KD, 512], F32R, tag="xts")
                nc.sync.dma_start(
                    out=xts[:, :, :nt],
                    in_=xgT[:, :, t0 : t0 + nt].rearrange("k p n -> p k n"),
                )

                # ---- phase A: h[m] = gelu(sum_k w1[k,m]^T x[k] + b1[m]) ----
                h = hpool.tile([128, MF, 512], F32R, tag="h")
                for m in range(MF):
                    w1sb = wpool.tile([128, KD, 128], F32R, tag="w1sb")
                    nc.sync.dma_start(out=w1sb, in_=w1tt[m])
                    psum1 = ps1.tile([128, 512], F32, tag="psum1")
                    for k in range(KD):
                        nc.tensor.matmul(
                            psum1[:, :nt],
                            lhsT=w1sb[:, k, :],
                            rhs=xts[:, k, :nt],
                            start=(k == 0),
                            stop=(k == KD - 1),
                        )
                    nc.scalar.activation(
                        h[:, m, :nt],
                        psum1[:, :nt],
                        AF.Gelu,
                        bias=b1sb[:, m : m + 1],
                    )

                # ---- phase B: y[dq] = sum_m w2[m,dq]^T h[m], one dq sweep ----
                for dq in range(DD):
                    w2sb = w2pool.tile([128, MF, 128], F32R, tag="w2sb")
                    nc.sync.dma_start(
                        out=w2sb[:, : MF // 2, :], in_=w2tt[dq, :, : MF // 2, :]
                    )
                    nc.sync.dma_start(
                        out=w2sb[:, MF // 2 :, :], in_=w2tt[dq, :, MF // 2 :, :]
                    )
                    psum2 = ps2.tile([128, 512], F32, tag="psum2")
                    for m in range(MF):
                        nc.tensor.matmul(
                            psum2[:, :nt],
                            lhsT=w2sb[:, m, :],
                            rhs=h[:, m, :nt],
                            start=(m == 0),
                            stop=(m == MF - 1),
                        )
                    y_sb = ypool.tile([128, 512], F32, tag="y_sb")
                    nc.vector.tensor_copy(y_sb[:, :nt], psum2[:, :nt])
                    nc.sync.dma_start(
                        out=yT[dq, :, t0 : t0 + nt],
                        in_=y_sb[:, :nt],
                    )
                t0 += nt

    nc.compile()
    return nc


def _get_program(C):
    if C not in _CACHE:
        _CACHE[C] = _build(C)
    return _CACHE[C]


def kernel(x, gate_w, gate_b, w1, b1, w2, b2):
    from concourse.bass_utils import run_bass_kernel_spmd

    x = np.asarray(x, dtype=np.float32)
    gate_w = np.asarray(gate_w, dtype=np.float32)
    gate_b = np.asarray(gate_b, dtype=np.float32)
    w1 = np.asarray(w1, dtype=np.float32)
    b1 = np.asarray(b1, dtype=np.float32)
    w2 = np.asarray(w2, dtype=np.float32)
    b2 = np.asarray(b2, dtype=np.float32)

    B, S, D = x.shape
    T = B * S
    xf = np.ascontiguousarray(x.reshape(T, D))

    # ---- gate (host): logits, top-2, softmax over the 2, aux loss ----
    logits = xf @ gate_w + gate_b                       # [T, E] f32
    sel = np.argsort(-logits, axis=-1, kind="stable")[:, :TOP_K]
    tl = np.take_along_axis(logits, sel, axis=1).astype(np.float64)
    ex = np.exp(tl - tl.max(axis=1, keepdims=True))
    topw = (ex / ex.sum(axis=1, keepdims=True)).astype(np.float32)

    counts = np.bincount(sel.ravel(), minlength=NUM_EXPERTS)
    fraction = counts.astype(np.float64) / (T * TOP_K)
    l64 = logits.astype(np.float64)
    sm = np.exp(l64 - l64.max(axis=1, keepdims=True))
    sm /= sm.sum(axis=1, keepdims=True)
    mean_prob = sm.mean(axis=0)
    aux_loss = np.float32(NUM_EXPERTS * np.sum(fraction * mean_prob))

    # ---- dispatch: gather each expert's tokens, pad to capacity C ----
    C = max(256, int(-(-counts.max() // 256)) * 256)
    nc = _get_program(C)

    sel0 = sel[:, 0]
    routing = []
    in_maps = []
    for e in range(NUM_EXPERTS):
        m0 = sel0 == e
        m_any = m0 | (sel[:, 1] == e)
        idx = np.nonzero(m_any)[0]
        g = np.where(m0[idx], topw[idx, 0], topw[idx, 1]).astype(np.float32)
        routing.append((idx, g))

        xg = np.zeros((C, D_MODEL), dtype=np.float32)
        xg[: len(idx)] = xf[idx]
        xgT = np.ascontiguousarray(xg.T).reshape(KD, 128, C)

        w1tt = np.ascontiguousarray(
            w1[e].reshape(KD, 128, MF, 128).transpose(2, 1, 0, 3)
        )
        w2tt = np.ascontiguousarray(
            w2[e].reshape(MF, 128, DD, 128).transpose(2, 1, 0, 3)
        )
        b1t = np.ascontiguousarray(b1[e].reshape(MF, 128).T)
        in_maps.append({"xgT": xgT, "w1tt": w1tt, "w2tt": w2tt, "b1t": b1t})

    try:
        res = run_bass_kernel_spmd(nc, in_maps, list(range(N_CORES)))
    except Exception:
        # Transient device wedges (NRT_EXEC_*) usually clear on retry.
        import time

        time.sleep(5)
        res = run_bass_kernel_spmd(nc, in_maps, list(range(N_CORES)))

    # ---- unshard: weighted scatter-add (+ b2), experts in ascending order ----
    out = np.zeros((T, D_MODEL), dtype=np.float32)
    for e in range(NUM_EXPERTS):
        idx, g = routing[e]
        y = res.results[e]["yT"].reshape(D_MODEL, C)[:, : len(idx)].T
        out[idx] += g[:, None] * (y + b2[e])

    return out.reshape(B, S, D_MODEL), aux_loss
